# revision 3
# baseline (speedup 1.0000x reference)
"""IndRNN kernel for 8 Trainium2 NeuronCores.

Math: h_t = relu(xw_t + b + u * h_{t-1}), h_0 = ones.  Output all h_t.

Strategy (v2: exact two-scan formulation)
-----------------------------------------
Closed form of the relu recurrence (exact, no approximation):

    p_t = u * p_{t-1} + xw_t,          p_0 = 0        (affine scan)
    q_t = min(u * q_{t-1}, p_t),       q_0 = -h_0     (min scan)
    h_t = relu(p_t - u * q_{t-1})

(The q scan tracks the best "relu restart point": unrolling h_t gives
max(0, max_s sum_{i=s..t} u^{t-i} xw_i, u^t h_0 + sum_{i=1..t} ...) and
every inner max-term equals p_t - u^{t-s} p_s; the min-scan computes the
min over those suffixes.)  Everything is O(max|xw|/(1-u)) -- no overflow,
valid for all u in (0,1), so no sorting/splitting of hidden units.

Mapping (per core: 4 batch rows x 2 hidden halves = 8 tiles [128, T]):
  - Tensor : xw = W^T x^T in PSUM per 512-col chunk (bf16 in, fp32 acc),
             bias b added via an extra ones-row matmul.
  - Scalar : PSUM -> SBUF copy; final relu(-(s)) with scale=-1 -> bf16.
  - Vector : the two scans (tensor_tensor_scan mult/add and mult/min).
  - GpSimd : s = u*q_{t-1} - p  (scalar_tensor_tensor mult/subtract).
  - I/O in bf16 (x pre-transposed/cast on host, out cast back on host);
    halves HBM traffic; rel-err ~0.4% << 2e-2 gate.
"""

import sys

for _p in ("/opt/trn_rl_repo",):
    if _p not in sys.path:
        sys.path.insert(0, _p)

from contextlib import ExitStack

import numpy as np
import ml_dtypes

import concourse.bass as bass
import concourse.tile as tile
from concourse import bacc, mybir
from concourse.bass_utils import run_bass_kernel_spmd

F32 = mybir.dt.float32
BF16 = mybir.dt.bfloat16
ALU = mybir.AluOpType
ACTF = mybir.ActivationFunctionType

B, T, D, H = 32, 4096, 256, 256
NCORES = 8
BLOC = B // NCORES  # batch rows per core
CHUNK = 512         # matmul N-tile (one PSUM bank)


def _build(nc):
    xt_d = nc.declare_dram_parameter("xt", [BLOC, D, T], BF16, isOutput=False)
    w_d = nc.declare_dram_parameter("w", [D, H], BF16, isOutput=False)
    br_d = nc.declare_dram_parameter("brow", [1, H], BF16, isOutput=False)
    uc_d = nc.declare_dram_parameter("ucol", [H, 1], F32, isOutput=False)
    out_d = nc.declare_dram_parameter("out", [BLOC, H, T], BF16, isOutput=True)

    nchunks = T // CHUNK

    with tile.TileContext(nc) as tc, ExitStack() as ctx:
        const = ctx.enter_context(tc.tile_pool(name="const", bufs=1))
        xt_pool = ctx.enter_context(tc.tile_pool(name="xt", bufs=2))
        psum_pool = ctx.enter_context(
            tc.tile_pool(name="psum", bufs=6, space=bass.MemorySpace.PSUM)
        )
        xw_pool = ctx.enter_context(tc.tile_pool(name="xw", bufs=2))
        p_pool = ctx.enter_context(tc.tile_pool(name="p", bufs=2))
        q_pool = ctx.enter_context(tc.tile_pool(name="q", bufs=2))
        s_pool = ctx.enter_context(tc.tile_pool(name="s", bufs=2))
        h_pool = ctx.enter_context(tc.tile_pool(name="h", bufs=2))

        # persistent weights / tables
        w_sb = []
        for dh in range(2):
            wt = const.tile([128, H], BF16, tag=f"w{dh}")
            nc.sync.dma_start(wt[:, :], w_d[dh * 128 : (dh + 1) * 128, :])
            w_sb.append(wt)
        brow_sb = const.tile([1, H], BF16, tag="brow")
        nc.sync.dma_start(brow_sb[:, :], br_d[:, :])
        ucol_sb = []
        for hh in range(2):
            ut = const.tile([128, 1], F32, tag=f"u{hh}")
            nc.sync.dma_start(ut[:, :], uc_d[hh * 128 : (hh + 1) * 128, :])
            ucol_sb.append(ut)
        ones_sb = const.tile([1, T], BF16, tag="ones")
        nc.vector.memset(ones_sb[:, :], 1.0)

        for b in range(BLOC):
            xts = []
            for dh in range(2):
                xtt = xt_pool.tile([128, T], BF16, tag=f"xt{dh}")
                nc.sync.dma_start(
                    xtt[:, :], xt_d[b, dh * 128 : (dh + 1) * 128, :]
                )
                xts.append(xtt)
            for hh in range(2):
                hsl = slice(hh * 128, (hh + 1) * 128)
                u_bc = ucol_sb[hh][:, 0:1].broadcast_to([128, T])

                xwb = xw_pool.tile([128, T], F32, tag="xw")
                for c in range(nchunks):
                    c0 = c * CHUNK
                    ps = psum_pool.tile([128, CHUNK], F32, tag="ps")
                    for dh in range(2):
                        nc.tensor.matmul(
                            ps[:, :],
                            w_sb[dh][:, hsl],
                            xts[dh][:, c0 : c0 + CHUNK],
                            start=(dh == 0),
                            stop=False,
                        )
                    nc.tensor.matmul(
                        ps[:, :],
                        brow_sb[:, hsl],
                        ones_sb[:, c0 : c0 + CHUNK],
                        start=False,
                        stop=True,
                    )
                    nc.scalar.activation(
                        xwb[:, c0 : c0 + CHUNK], ps[:, :], ACTF.Copy
                    )

                # p_t = u p_{t-1} + xw_t   (state starts at p_0 = 0)
                p = p_pool.tile([128, T], F32, tag="p")
                nc.vector.tensor_tensor_scan(
                    p[:, :], u_bc, xwb[:, :], 0.0, op0=ALU.mult, op1=ALU.add
                )
                # q_t = min(u q_{t-1}, p_t),  q_0 = -h_0 = -1
                q = q_pool.tile([128, T + 1], F32, tag="q")
                nc.vector.memset(q[:, 0:1], -1.0)
                nc.vector.tensor_tensor_scan(
                    q[:, 1 : T + 1], u_bc, p[:, :], -1.0,
                    op0=ALU.mult, op1=ALU.min,
                )
                # s1 = u*q_{t-1} ;  s = s1 - p_t ;  h = relu(-s)
                s1 = s_pool.tile([128, T], BF16, tag="s1")
                nc.gpsimd.tensor_scalar(
                    s1[:, :], q[:, 0:T], ucol_sb[hh][:, :], None, op0=ALU.mult
                )
                s = s_pool.tile([128, T], BF16, tag="s")
                nc.gpsimd.tensor_tensor(
                    s[:, :], s1[:, :], p[:, :], op=ALU.subtract
                )
                h = h_pool.tile([128, T], BF16, tag="h")
                nc.scalar.activation(h[:, :], s[:, :], ACTF.Relu, scale=-1.0)
                nc.sync.dma_start(out_d[b, hsl, :], h[:, :])


def _host_prep(x, W, b, u):
    x = np.asarray(x, np.float32)
    W = np.asarray(W, np.float32)
    b = np.asarray(b, np.float32)
    u = np.asarray(u, np.float32)

    xt = np.ascontiguousarray(
        np.swapaxes(x, 1, 2).astype(ml_dtypes.bfloat16)
    )  # [B, D, T] bf16
    common = {
        "w": np.ascontiguousarray(W.astype(ml_dtypes.bfloat16)),
        "brow": np.ascontiguousarray(b[None, :].astype(ml_dtypes.bfloat16)),
        "ucol": np.ascontiguousarray(u[:, None]),
    }
    in_maps = []
    for c in range(NCORES):
        m = dict(common)
        m["xt"] = np.ascontiguousarray(xt[c * BLOC : (c + 1) * BLOC])
        in_maps.append(m)
    return in_maps


# set by test harnesses to profile: kernel() stores the raw results here
LAST_RESULT = None


def kernel(x, W, b, u):
    global LAST_RESULT
    import os

    in_maps = _host_prep(x, W, b, u)

    nc = bacc.Bacc("TRN2", target_bir_lowering=False, debug=False)
    _build(nc)
    nc.compile()

    trace = bool(os.environ.get("INDRNN_TRACE"))
    res = run_bass_kernel_spmd(
        nc, in_maps, core_ids=list(range(NCORES)), trace=trace
    )
    LAST_RESULT = res
    out_dev = np.concatenate(
        [np.asarray(r["out"]) for r in res.results], axis=0
    )  # [B, H, T] bf16

    out = np.ascontiguousarray(
        np.swapaxes(out_dev, 1, 2).astype(np.float32)
    )  # [B, T, H] fp32
    return out


# revision 7
# speedup vs baseline: 2.8317x; 2.8317x over previous
"""IndRNN kernel for 8 Trainium2 NeuronCores.

Math: h_t = relu(xw_t + b + u * h_{t-1}), h_0 = ones.  Output all h_t.

Strategy (v2: exact two-scan formulation)
-----------------------------------------
Closed form of the relu recurrence (exact, no approximation):

    p_t = u * p_{t-1} + xw_t,          p_0 = 0        (affine scan)
    q_t = min(u * q_{t-1}, p_t),       q_0 = -h_0     (min scan)
    h_t = relu(p_t - u * q_{t-1})

(The q scan tracks the best "relu restart point": unrolling h_t gives
max(0, max_s sum_{i=s..t} u^{t-i} xw_i, u^t h_0 + sum_{i=1..t} ...) and
every inner max-term equals p_t - u^{t-s} p_s; the min-scan computes the
min over those suffixes.)  Everything is O(max|xw|/(1-u)) -- no overflow,
valid for all u in (0,1), so no sorting/splitting of hidden units.

Mapping (per core: 4 batch rows x 2 hidden halves = 8 tiles [128, T]):
  - Tensor : xw = W^T x^T in PSUM per 512-col chunk (bf16 in, fp32 acc),
             bias b added via an extra ones-row matmul.
  - Scalar : PSUM -> SBUF copy; final relu(-(s)) with scale=-1 -> bf16.
  - Vector : the two scans (tensor_tensor_scan mult/add and mult/min).
  - GpSimd : s = u*q_{t-1} - p  (scalar_tensor_tensor mult/subtract).
  - I/O in bf16 (x pre-transposed/cast on host, out cast back on host);
    halves HBM traffic; rel-err ~0.4% << 2e-2 gate.
"""

import sys

for _p in ("/opt/trn_rl_repo",):
    if _p not in sys.path:
        sys.path.insert(0, _p)

from contextlib import ExitStack

import numpy as np
import ml_dtypes

import concourse.bass as bass
import concourse.tile as tile
from concourse import bacc, mybir
from concourse.bass_utils import run_bass_kernel_spmd

F32 = mybir.dt.float32
BF16 = mybir.dt.bfloat16
ALU = mybir.AluOpType
ACTF = mybir.ActivationFunctionType

B, T, D, H = 32, 4096, 256, 256
NCORES = 8
BLOC = B // NCORES  # batch rows per core
CHUNK = 512         # matmul N-tile (one PSUM bank)


def _build(nc):
    xt_d = nc.declare_dram_parameter("xt", [BLOC, D, T], BF16, isOutput=False)
    w_d = nc.declare_dram_parameter("w", [D, H], BF16, isOutput=False)
    br_d = nc.declare_dram_parameter("brow", [1, H], BF16, isOutput=False)
    uc_d = nc.declare_dram_parameter("ucol", [H, 1], F32, isOutput=False)
    nu_d = nc.declare_dram_parameter("nucol", [H, 1], F32, isOutput=False)
    out_d = nc.declare_dram_parameter("out", [BLOC, H, T], BF16, isOutput=True)

    nchunks = T // CHUNK

    with tile.TileContext(nc) as tc, ExitStack() as ctx:
        const = ctx.enter_context(tc.tile_pool(name="const", bufs=1))
        xt_pool = ctx.enter_context(tc.tile_pool(name="xt", bufs=2))
        psum_pool = ctx.enter_context(
            tc.tile_pool(name="psum", bufs=6, space=bass.MemorySpace.PSUM)
        )
        xw_pool = ctx.enter_context(tc.tile_pool(name="xw", bufs=2))
        p_pool = ctx.enter_context(tc.tile_pool(name="p", bufs=2))
        q_pool = ctx.enter_context(tc.tile_pool(name="q", bufs=2))
        s_pool = ctx.enter_context(tc.tile_pool(name="s", bufs=2))
        h_pool = ctx.enter_context(tc.tile_pool(name="h", bufs=2))

        # persistent weights / tables
        w_sb = []
        for dh in range(2):
            wt = const.tile([128, H], BF16, tag=f"w{dh}")
            nc.sync.dma_start(wt[:, :], w_d[dh * 128 : (dh + 1) * 128, :])
            w_sb.append(wt)
        brow_sb = const.tile([1, H], BF16, tag="brow")
        nc.sync.dma_start(brow_sb[:, :], br_d[:, :])
        ucol_sb = []
        nucol_sb = []
        for hh in range(2):
            ut = const.tile([128, 1], F32, tag=f"u{hh}")
            nc.sync.dma_start(ut[:, :], uc_d[hh * 128 : (hh + 1) * 128, :])
            ucol_sb.append(ut)
            nt = const.tile([128, 1], F32, tag=f"nu{hh}")
            nc.sync.dma_start(nt[:, :], nu_d[hh * 128 : (hh + 1) * 128, :])
            nucol_sb.append(nt)
        ones_sb = const.tile([1, T], BF16, tag="ones")
        nc.vector.memset(ones_sb[:, :], 1.0)

        for b in range(BLOC):
            xts = []
            for dh in range(2):
                xtt = xt_pool.tile([128, T], BF16, tag=f"xt{dh}")
                nc.sync.dma_start(
                    xtt[:, :], xt_d[b, dh * 128 : (dh + 1) * 128, :]
                )
                xts.append(xtt)
            for hh in range(2):
                hsl = slice(hh * 128, (hh + 1) * 128)
                u_bc = ucol_sb[hh][:, 0:1].broadcast_to([128, T])

                xwb = xw_pool.tile([128, T], F32, tag="xw")
                for c in range(nchunks):
                    c0 = c * CHUNK
                    ps = psum_pool.tile([128, CHUNK], F32, tag="ps")
                    for dh in range(2):
                        nc.tensor.matmul(
                            ps[:, :],
                            w_sb[dh][:, hsl],
                            xts[dh][:, c0 : c0 + CHUNK],
                            start=(dh == 0),
                            stop=False,
                        )
                    nc.tensor.matmul(
                        ps[:, :],
                        brow_sb[:, hsl],
                        ones_sb[:, c0 : c0 + CHUNK],
                        start=False,
                        stop=True,
                    )
                    nc.scalar.activation(
                        xwb[:, c0 : c0 + CHUNK], ps[:, :], ACTF.Copy
                    )

                # p_t = u p_{t-1} + xw_t   (state starts at p_0 = 0)
                p = p_pool.tile([128, T], F32, tag="p")
                nc.vector.tensor_tensor_scan(
                    p[:, :], u_bc, xwb[:, :], 0.0, op0=ALU.mult, op1=ALU.add
                )
                # r_t = u*q_t via state' = u*min(p_t, state); r col j = u*q_j
                r = q_pool.tile([128, T + 1], F32, tag="r")
                nc.vector.tensor_copy(r[:, 0:1], nucol_sb[hh][:, :])
                nc.vector.tensor_tensor_scan(
                    r[:, 1 : T + 1], p[:, :], u_bc, nucol_sb[hh][:, 0:1],
                    op0=ALU.min, op1=ALU.mult,
                )
                # s = u*q_{t-1} - p_t  (gpsimd, all-fp32 fast path)
                s = s_pool.tile([128, T], F32, tag="s")
                nc.gpsimd.tensor_tensor(
                    s[:, :], r[:, 0:T], p[:, :], op=ALU.subtract
                )
                # h = relu(-s) = relu(p - u*q_{t-1})
                h = h_pool.tile([128, T], BF16, tag="h")
                nc.scalar.activation(h[:, :], s[:, :], ACTF.Relu, scale=-1.0)
                nc.sync.dma_start(out_d[b, hsl, :], h[:, :])


def _host_prep(x, W, b, u):
    x = np.asarray(x, np.float32)
    W = np.asarray(W, np.float32)
    b = np.asarray(b, np.float32)
    u = np.asarray(u, np.float32)

    xt = np.ascontiguousarray(
        np.swapaxes(x, 1, 2).astype(ml_dtypes.bfloat16)
    )  # [B, D, T] bf16
    common = {
        "w": np.ascontiguousarray(W.astype(ml_dtypes.bfloat16)),
        "brow": np.ascontiguousarray(b[None, :].astype(ml_dtypes.bfloat16)),
        "ucol": np.ascontiguousarray(u[:, None]),
        "nucol": np.ascontiguousarray(-u[:, None]),
    }
    in_maps = []
    for c in range(NCORES):
        m = dict(common)
        m["xt"] = np.ascontiguousarray(xt[c * BLOC : (c + 1) * BLOC])
        in_maps.append(m)
    return in_maps


# set by test harnesses to profile: kernel() stores the raw results here
LAST_RESULT = None


def kernel(x, W, b, u):
    global LAST_RESULT
    import os

    in_maps = _host_prep(x, W, b, u)

    nc = bacc.Bacc("TRN2", target_bir_lowering=False, debug=False)
    _build(nc)
    nc.compile()

    trace = bool(os.environ.get("INDRNN_TRACE"))
    res = run_bass_kernel_spmd(
        nc, in_maps, core_ids=list(range(NCORES)), trace=trace
    )
    LAST_RESULT = res
    out_dev = np.concatenate(
        [np.asarray(r["out"]) for r in res.results], axis=0
    )  # [B, H, T] bf16

    out = np.ascontiguousarray(
        np.swapaxes(out_dev, 1, 2).astype(np.float32)
    )  # [B, T, H] fp32
    return out
